# revision 16
# baseline (speedup 1.0000x reference)
"""Trainium2 Bass kernel for CapsuleLayer dynamic routing (v5).

Problem: x [64, 2048, 16], W [1, 2048, 32, 32, 16] ->
  u_hat = einsum('bik,ijdk->bijd', x, W[0])           [B, N_in, N_out, D_out]
  3 rounds of routing (softmax over j, weighted sum over i, squash),
  returns v [64, 32, 32].

Sharding: N_in (2048) split over 8 cores, 256 local capsules each; per-round
partial weighted sums AllReduced; softmax/squash replicated.

v5 redesign (vs the v2 baseline at 745us):
  * Batch in 4 chunks of 16; chunk round-chains are interleaved two at a
    time so every AllReduce's latency hides under the other chunk's DVE
    work (U double-buffered at 64KB/partition per chunk).
  * phase 1 emits u_hat with M=128 PSUM rows ((e8,b16) out partitions,
    K=(e8,k16)=128 zero-interleaved x stationary, W streamed as the
    moving operand): 4x less PE time than the 32-row-tile baseline.
  * The agreement's reduce over d is NOT a DVE tree: the PE accumulates
    the 32 d-slices of the product P=U*v into logits PSUM through an
    identity stationary (f32 accumulation, frees ~130us of DVE).
  * Weighted-sum keeps the ones-matmul i-reduction; products (U*v, U*c)
    are the only big DVE work left, at 2x_1p bf16.
  * All PSUM->SBUF drains on ACT; AllReduce machinery + replicate DMAs
    on the Pool queue (interleaved in dependency order); W streamed once
    per chunk on the sync queue.
"""
import sys

sys.path.insert(0, '/opt/trn_rl_repo')

import numpy as np

import concourse.bass as bass
import concourse.mybir as mybir
from concourse import bass_utils, tile

# ---------------------------------------------------------------- constants
N_CORES = 8
B = 64
N_IN = 2048
D_IN = 16
N_OUT = 32
D_OUT = 32
EPS = 1e-9

I_LOC = N_IN // N_CORES          # 256 local capsules
NG = 32                          # capsule groups of 8 (i = g*8 + e)
BC = 16                          # batch chunk
NCHUNK = B // BC                 # 4
JD = N_OUT * D_OUT               # 1024 (d,j) values per capsule
UCOLS = NG * JD                  # 32768 U columns per chunk
SECG = 4                         # capsule groups per section
NSEC = NG // SECG                # 8 sections per chunk
SEC_COLS = SECG * JD             # 4096

f32 = mybir.dt.float32
bf16 = mybir.dt.bfloat16

_MAX_WAITS = 1
_carrier = [0]


def _patch_tile():
    """Work around this walrus build rejecting >1 sync wait per instruction."""
    import concourse.mybir as _mybir
    from concourse import tile as _tile
    from concourse.tile import TileContext as _TC

    def _drain_and_barrier(self, tick_clock, wait_clock):
        ScopedClock = _tile.ScopedClock
        probe = self.nc.sync.nop(nofuse=True)
        wait_clock.add_sem_waits(
            probe.ins, ScopedClock({None: tick_clock.global_clock})
        )
        si = probe.ins.sync_info
        waits = list(si.on_wait)
        probe.ins.sync_info = _mybir.SyncInfo(
            on_wait=waits[:1], on_update=list(si.on_update)
        )
        for w in waits[1:]:
            carrier = self.nc.sync.nop(nofuse=True)
            carrier.ins.sync_info = _mybir.SyncInfo(on_wait=[w], on_update=[])
        self.nc.sync.drain()
        self.nc.all_engine_barrier()
        assert self.sems is not None
        popped = self.nc._tile_sem_poison_stack.pop()
        assert popped is self._sem_poison
        self.nc.clear_and_free_semaphores(list(self.sems.allocated().values()))
        self.nc.all_engine_barrier()

    _TC._drain_and_barrier = _drain_and_barrier

    try:
        from concourse import tile_utils
        tile_utils.max_sbuf_usage = 208 * 1024
    except Exception:
        pass


def _fix_sync_waits(nc, max_waits=_MAX_WAITS):
    n_fixed = 0
    for func in nc.m.functions:
        for bb in func.blocks:
            insts = list(bb.instructions)
            new_list = []
            changed = False
            for inst in insts:
                si = getattr(inst, "sync_info", None)
                waits = list(si.on_wait) if si is not None else []
                if len(waits) > max_waits:
                    keep = waits[: max_waits - 1] if max_waits > 1 else []
                    hoist = waits[len(keep):-1]
                    tail = [waits[-1]]
                    for w in hoist:
                        _carrier[0] += 1
                        nop = mybir.InstNoOp(
                            name=f"syncfix-{_carrier[0]}", engine=inst.engine
                        )
                        nop.sync_info = mybir.SyncInfo(on_wait=[w], on_update=[])
                        new_list.append(nop)
                    inst.sync_info = mybir.SyncInfo(
                        on_wait=keep + tail, on_update=list(si.on_update)
                    )
                    changed = True
                    n_fixed += 1
                new_list.append(inst)
            if changed:
                bb.instructions = new_list
    return n_fixed


# ---------------------------------------------------------------- program
def _build_program():
    _patch_tile()
    nc = bass.Bass(trn_type="TRN2", num_devices=N_CORES)

    wt_in = nc.dram_tensor("wt", [128, UCOLS], bf16, kind="ExternalInput")
    xin_in = nc.dram_tensor("xin", [128, NCHUNK * NG * 128], bf16,
                            kind="ExternalInput")
    xd_in = nc.dram_tensor("xd", [128, NG * B], bf16, kind="ExternalInput")
    id_in = nc.dram_tensor("ident", [128, 128], bf16, kind="ExternalInput")
    ones_in = nc.dram_tensor("ones16", [128, BC], bf16, kind="ExternalInput")
    v_out = nc.dram_tensor("v", [B, JD], f32, kind="ExternalOutput")

    AluOp = mybir.AluOpType
    Act = mybir.ActivationFunctionType
    Axis = mybir.AxisListType
    rg = [list(range(N_CORES))]

    from contextlib import ExitStack
    with tile.TileContext(nc, num_cores=N_CORES) as tc, ExitStack() as es:
        cpool = es.enter_context(tc.tile_pool(name="const", bufs=1))
        wpool = es.enter_context(tc.tile_pool(name="wstream", bufs=4))
        upool = es.enter_context(tc.tile_pool(name="ubuf", bufs=1))
        scpool = es.enter_context(tc.tile_pool(name="scratch", bufs=1))
        smpool = es.enter_context(tc.tile_pool(name="small", bufs=1))
        psph = es.enter_context(tc.tile_pool(name="psph1", bufs=2, space="PSUM"))
        pslg = es.enter_context(tc.tile_pool(name="pslog", bufs=1, space="PSUM"))
        psws = es.enter_context(tc.tile_pool(name="psws", bufs=1, space="PSUM"))
        dpool = es.enter_context(tc.tile_pool(name="dram", bufs=1, space="DRAM"))

        # ---- constants / inputs resident in SBUF
        ident = cpool.tile([128, 128], bf16, tag="ident")
        ones16 = cpool.tile([128, BC], bf16, tag="ones16")
        xd = cpool.tile([128, NG * B], bf16, tag="xd")
        nc.sync.dma_start(ident[:], id_in[:])
        nc.sync.dma_start(ones16[:], ones_in[:])
        nc.sync.dma_start(xd[:], xd_in[:])

        # ---- big buffers
        U = [upool.tile([128, UCOLS], bf16, tag=f"U{h}", name=f"U{h}")
             for h in range(2)]
        xint = [cpool.tile([128, NG * 128], bf16, tag=f"xint{h}",
                           name=f"xint{h}") for h in range(2)]
        pbuf = scpool.tile([128, 2 * SEC_COLS], bf16, tag="pbuf")
        slots = [pbuf[:, :SEC_COLS], pbuf[:, SEC_COLS:]]
        algA = [smpool.tile([128, NG * N_OUT], f32, tag=f"algA{h}",
                            name=f"algA{h}") for h in range(2)]
        cbuf = [smpool.tile([128, NG * N_OUT], bf16, tag=f"c{h}",
                            name=f"c{h}") for h in range(2)]
        vt = [smpool.tile([128, JD], bf16, tag=f"vt{h}", name=f"vt{h}")
              for h in range(2)]
        ebuf = smpool.tile([128, NG * N_OUT], f32, tag="ebuf")
        s_rep = smpool.tile([128, JD], f32, tag="srep")
        v0all = smpool.tile([128, JD], bf16, tag="v0all")
        Zt = smpool.tile([128, NG], f32, tag="Zt")
        Zr = smpool.tile([128, NG], f32, tag="Zr")
        # aliases: squash's square scratch reuses ebuf; the softmax
        # Zr-replica reuses s_rep (lifetimes strictly serialized on DVE).
        p2 = ebuf
        zrep = s_rep
        s2 = smpool.tile([128, N_OUT], f32, tag="s2")
        lns = smpool.tile([128, N_OUT], f32, tag="lns")
        rsq = smpool.tile([128, N_OUT], f32, tag="rsq")
        den = smpool.tile([128, N_OUT], f32, tag="den")
        rinv = smpool.tile([128, N_OUT], f32, tag="rinv")
        fsc = smpool.tile([128, N_OUT], f32, tag="fsc")
        ibuf = smpool.tile([128, N_OUT], mybir.dt.int32, tag="ibuf")
        s_sb = smpool.tile([B, JD], f32, tag="ssb")
        v_fin = smpool.tile([BC, JD], f32, tag="vfin")

        # PSUM tiles
        ps_log = pslg.tile([128, NG * N_OUT], f32, tag="pslog")
        ps_ws = psws.tile([B, JD], f32, tag="psws")

        # AR dram staging
        dum_in = dpool.tile([16, 16], f32, tag="dumi")
        dum_out = dpool.tile([16, 16], f32, tag="dumo")
        ar0_in = dpool.tile([B, JD], f32, tag="ar0i")
        ar0_out = dpool.tile([B, JD], f32, tag="ar0o")
        ar_bufs = {}
        for q in range(NCHUNK):
            for t in (1, 2):
                ar_bufs[(q, t)] = (
                    dpool.tile([BC, JD], f32, name=f"ari{q}{t}"),
                    dpool.tile([BC, JD], f32, name=f"aro{q}{t}"),
                )

        # ------------------------------------------------------ phase 1
        def warmup():
            """Back-to-back full matmuls to ramp the PE p-state before the
            s0/phase-1 burst (output never read)."""
            ps = psph.tile([128, JD], f32, tag="ph1", name="warm")
            for i in range(16):
                nc.tensor.matmul(
                    ps[:, 0:512], xd[:, 0:128], xd[:, 0:512],
                    start=True, stop=True,
                )

        def phase1_fused01():
            """One W pass serving s0 (full batch) + phase1 of chunks 0 and
            1: per W tile, 2 s0-matmuls accumulate into ps_ws and 2+2
            matmuls fill U0/U1; c0 PSUMs drain on DVE (idle during S0),
            c1 on ACT."""
            nc.sync.dma_start(xint[0][:], xin_in[:, 0:NG * 128])
            nc.sync.dma_start(
                xint[1][:], xin_in[:, NG * 128:2 * NG * 128])
            for g in range(NG):
                w = wpool.tile([128, JD], bf16, tag="w")
                dmaq = nc.sync if g % 2 == 0 else nc.scalar
                dmaq.dma_start(w[:], wt_in[:, g * JD:(g + 1) * JD])
                psA = psph.tile([128, JD], f32, tag="ph1", name=f"psA{g}")
                psB = psph.tile([128, JD], f32, tag="ph1", name=f"psB{g}")
                for half in range(2):
                    nc.tensor.matmul(
                        psA[:, half * 512:(half + 1) * 512],
                        xint[0][:, g * 128:(g + 1) * 128],
                        w[:, half * 512:(half + 1) * 512],
                        start=True, stop=True,
                    )
                    nc.tensor.matmul(
                        psB[:, half * 512:(half + 1) * 512],
                        xint[1][:, g * 128:(g + 1) * 128],
                        w[:, half * 512:(half + 1) * 512],
                        start=True, stop=True,
                    )
                nc.vector.tensor_copy(U[0][:, g * JD:(g + 1) * JD], psA[:])
                nc.scalar.copy(U[1][:, g * JD:(g + 1) * JD], psB[:])
                for half in range(2):
                    nc.tensor.matmul(
                        ps_ws[:, half * 512:(half + 1) * 512],
                        xd[:, g * B:(g + 1) * B],
                        w[:, half * 512:(half + 1) * 512],
                        start=(g == 0), stop=(g == NG - 1),
                    )

        def phase1(q, g0=0, g1=NG, with_s0=False):
            """u_hat groups [g0,g1) for batch chunk q into U[q%2]; W
            streamed per group on the sync queue; PSUM drained on ACT.
            If with_s0, also accumulates s0 = sum_i u_hat/32 for the full
            batch into ps_ws through the dense xd stationary.  Mid-kernel
            chunks are emitted in two halves around the concurrent round's
            softmax so the ACT/PE FIFOs never head-of-line-block it."""
            Uq = U[q % 2]
            xq = xint[q % 2]
            if g0 == 0:
                nc.sync.dma_start(
                    xq[:], xin_in[:, q * NG * 128:(q + 1) * NG * 128])
            for g in range(g0, g1):
                w = wpool.tile([128, JD], bf16, tag="w")
                dmaq = nc.sync if g % 2 == 0 else nc.gpsimd
                dmaq.dma_start(w[:], wt_in[:, g * JD:(g + 1) * JD])
                ps = psph.tile([128, JD], f32, tag="ph1")
                for half in range(2):
                    if with_s0:
                        nc.tensor.matmul(
                            ps_ws[:, half * 512:(half + 1) * 512],
                            xd[:, g * B:(g + 1) * B],
                            w[:, half * 512:(half + 1) * 512],
                            start=(g == 0), stop=(g == NG - 1),
                        )
                    nc.tensor.matmul(
                        ps[:, half * 512:(half + 1) * 512],
                        xq[:, g * 128:(g + 1) * 128],
                        w[:, half * 512:(half + 1) * 512],
                        start=True, stop=True,
                    )
                nc.scalar.copy(Uq[:, g * JD:(g + 1) * JD], ps[:])

        # ------------------------------------------------------ routing ops
        def agreement(q, t):
            """logits psum[p=(e,b), (g,j)] = sum_d U*v via DVE product +
            PE identity-matmul accumulation over the 32 d slices."""
            Uq = U[q % 2]
            v4 = (vt[q % 2][:]
                  .rearrange("p (d j) -> p d j", d=D_OUT, j=N_OUT)
                  .unsqueeze(1)
                  .to_broadcast((128, SECG, D_OUT, N_OUT)))
            P2 = pbuf[:].rearrange(
                "p (g d j) -> p g d j", g=2 * SECG, d=D_OUT, j=N_OUT)
            for sec in range(NSEC):
                slot = slots[sec % 2]
                P = slot.rearrange(
                    "p (g d j) -> p g d j", g=SECG, d=D_OUT, j=N_OUT)
                Us = Uq[:, sec * SEC_COLS:(sec + 1) * SEC_COLS].rearrange(
                    "p (g d j) -> p g d j", g=SECG, d=D_OUT, j=N_OUT)
                nc.vector.tensor_tensor(P, Us, v4, AluOp.mult)   # 2x
                if sec % 2 == 1:
                    # one identity-matmul sweep reduces BOTH sections of
                    # the pair buffer (N=256: halves the LDWEIGHTS tax)
                    for dd in range(D_OUT):
                        nc.tensor.matmul(
                            ps_log[:, (sec - 1) * SECG * N_OUT:
                                   (sec + 1) * SECG * N_OUT],
                            ident[:],
                            P2[:, :, dd, :],
                            start=(dd == 0), stop=(dd == D_OUT - 1),
                        )

        def softmax(q, t):
            """c = softmax over j of logits (+ prev-round logits for t=2)."""
            A = algA[q % 2]
            if t == 1:
                nc.scalar.copy(A[:], ps_log[:])
            else:
                nc.vector.tensor_add(A[:], A[:], ps_log[:])
            nc.scalar.activation(ebuf[:], A[:], Act.Exp)
            e3 = ebuf[:].rearrange("p (g j) -> p g j", g=NG, j=N_OUT)
            nc.vector.reduce_sum(Zt[:], e3, axis=Axis.X)
            nc.vector.reciprocal(Zr[:], Zt[:])
            nc.vector.tensor_copy(
                zrep[:].rearrange("p (g j) -> p g j", g=NG, j=N_OUT),
                Zr[:].unsqueeze(2).to_broadcast((128, NG, N_OUT)))
            nc.vector.tensor_tensor(
                cbuf[q % 2][:], ebuf[:], zrep[:], AluOp.mult)

        def weighted_sum(q, t):
            """s_partial[b,(d,j)] = sum_i c*U: DVE product (2x) + PE
            ones-matmul reduction over (e-partitions, g-psum-accum)."""
            Uq = U[q % 2]
            cq = cbuf[q % 2]
            for sec in range(NSEC):
                slot = slots[sec % 2]
                P = slot[:].rearrange(
                    "p (g d j) -> p g d j", g=SECG, d=D_OUT, j=N_OUT)
                Us = Uq[:, sec * SEC_COLS:(sec + 1) * SEC_COLS].rearrange(
                    "p (g d j) -> p g d j", g=SECG, d=D_OUT, j=N_OUT)
                c4 = (cq[:, sec * SECG * N_OUT:(sec + 1) * SECG * N_OUT]
                      .rearrange("p (g j) -> p g j", g=SECG, j=N_OUT)
                      .unsqueeze(2)
                      .to_broadcast((128, SECG, D_OUT, N_OUT)))
                nc.vector.tensor_tensor(P, Us, c4, AluOp.mult)   # 2x
                for g in range(SECG):
                    for half in range(2):
                        nc.tensor.matmul(
                            ps_ws[0:BC, half * 512:(half + 1) * 512],
                            ones16[:],
                            slot[:, g * JD + half * 512:
                                 g * JD + half * 512 + 512],
                            start=(sec == 0 and g == 0),
                            stop=(sec == NSEC - 1 and g == SECG - 1),
                        )
            nc.scalar.copy(s_sb[0:BC, :], ps_ws[0:BC, :])
            ar_in, ar_out = ar_bufs[(q, t)]
            nc.gpsimd.dma_start(ar_in[:], s_sb[0:BC, :])
            nc.gpsimd.collective_compute(
                "AllReduce", AluOp.add, replica_groups=rg,
                ins=[ar_in.opt()], outs=[ar_out.opt()],
            )
            return ar_out

        def squash_core(rows, out_tt):
            """Common squash tail: rows = partition count holding s in
            s_rep; out_tt(s3, f3) emits the final multiply."""
            nc.scalar.square(p2[0:rows, :], s_rep[0:rows, :])
            p3 = p2[0:rows, :].rearrange("p (d j) -> p j d", d=D_OUT, j=N_OUT)
            nc.vector.reduce_sum(s2[0:rows, :], p3, axis=Axis.X)
            nc.vector.tensor_scalar_add(den[0:rows, :], s2[0:rows, :],
                                        1.0 + EPS)
            nc.vector.tensor_scalar_add(lns[0:rows, :], s2[0:rows, :], EPS)
            ii = lns[0:rows, :].bitcast(mybir.dt.int32)
            nc.vector.tensor_scalar(
                ibuf[0:rows, :], ii, 1, None,
                mybir.AluOpType.logical_shift_right)
            nc.vector.tensor_scalar(
                ibuf[0:rows, :], ibuf[0:rows, :], 0x5F3759DF, -1,
                mybir.AluOpType.subtract, mybir.AluOpType.mult)
            y0 = ibuf[0:rows, :].bitcast(f32)
            nc.vector.tensor_mul(rsq[0:rows, :], y0, y0)
            nc.vector.tensor_mul(rsq[0:rows, :], rsq[0:rows, :],
                                 lns[0:rows, :])
            nc.vector.tensor_scalar(
                rsq[0:rows, :], rsq[0:rows, :], -0.5, 1.5,
                mybir.AluOpType.mult, mybir.AluOpType.add)
            nc.vector.tensor_mul(rsq[0:rows, :], rsq[0:rows, :], y0)
            nc.vector.tensor_mul(fsc[0:rows, :], rsq[0:rows, :],
                                 rsq[0:rows, :])
            nc.vector.tensor_mul(fsc[0:rows, :], fsc[0:rows, :],
                                 lns[0:rows, :])
            nc.vector.tensor_scalar(
                fsc[0:rows, :], fsc[0:rows, :], -0.5, 1.5,
                mybir.AluOpType.mult, mybir.AluOpType.add)
            nc.vector.tensor_mul(rsq[0:rows, :], rsq[0:rows, :],
                                 fsc[0:rows, :])
            nc.vector.reciprocal(rinv[0:rows, :], den[0:rows, :])
            nc.vector.tensor_mul(fsc[0:rows, :], rsq[0:rows, :],
                                 rinv[0:rows, :])
            nc.vector.tensor_mul(fsc[0:rows, :], fsc[0:rows, :],
                                 s2[0:rows, :])
            out_tt()

        def squash_v0():
            """v0 = squash(AllReduce(s0)) for the full batch at
            [p=(rep2,b64)]."""
            for r in range(2):
                nc.gpsimd.dma_start(
                    s_rep[64 * r:64 * r + 64, :], ar0_out[:])

            def tt():
                s3 = s_rep[:].rearrange("p (d j) -> p d j", d=D_OUT, j=N_OUT)
                f3 = fsc[:].unsqueeze(1).to_broadcast((128, D_OUT, N_OUT))
                v3 = v0all[:].rearrange("p (d j) -> p d j", d=D_OUT, j=N_OUT)
                nc.vector.tensor_tensor(v3, s3, f3, AluOp.mult)
            squash_core(128, tt)

        def rep_v0(q):
            """vt[q%2] <- per-chunk (e8,b16)-replicated slice of v0all."""
            dst = vt[q % 2]
            for e in range(8):
                nc.gpsimd.dma_start(
                    dst[16 * e:16 * e + 16, :],
                    v0all[q * BC:(q + 1) * BC, :])

        def rep_s(ar_out):
            """Replicate an AllReduced [16,1024] s into s_rep's 8 e-groups
            (Pool queue; emit right after the producing collective so the
            queue never head-blocks)."""
            for e in range(8):
                nc.gpsimd.dma_start(
                    s_rep[16 * e:16 * e + 16, :], ar_out[:])

        def squash_round(q, ar_out, reps_done=False):
            """v_{t} for chunk q from its AllReduced s, into vt[q%2]."""
            if not reps_done:
                rep_s(ar_out)

            def tt():
                s3 = s_rep[:].rearrange("p (d j) -> p d j", d=D_OUT, j=N_OUT)
                f3 = fsc[:].unsqueeze(1).to_broadcast((128, D_OUT, N_OUT))
                v3 = vt[q % 2][:].rearrange(
                    "p (d j) -> p d j", d=D_OUT, j=N_OUT)
                nc.vector.tensor_tensor(v3, s3, f3, AluOp.mult)
            squash_core(128, tt)

        def squash_final(q, ar_out):
            """Final v for chunk q -> v_out rows, reference layout."""
            for e in range(8):
                nc.gpsimd.dma_start(
                    s_rep[16 * e:16 * e + 16, :], ar_out[:])

            def tt():
                vf = v_fin[:].rearrange("p (j d) -> p d j", j=N_OUT, d=D_OUT)
                nc.vector.tensor_tensor(
                    vf,
                    s_rep[0:BC, :].rearrange(
                        "p (d j) -> p d j", d=D_OUT, j=N_OUT),
                    fsc[0:BC, :].unsqueeze(1).to_broadcast(
                        (BC, D_OUT, N_OUT)),
                    AluOp.mult)
                nc.gpsimd.dma_start(v_out[q * BC:(q + 1) * BC, :], v_fin[:])
            squash_core(BC, tt)

        def round_(q, t):
            agreement(q, t)
            softmax(q, t)
            return weighted_sum(q, t)

        # ------------------------------------------------------ emission
        # S0: warm PE, stream W once for chunk 0 while accumulating s0 for
        # the full batch; AllReduce s0; squash v0.
        # CC cold-start warmup: a tiny AllReduce enqueued at t=0 absorbs
        # the ~40us first-collective spin-up while S0 computes.
        nc.gpsimd.dma_start(dum_in[:], s_sb[0:16, 0:16])
        nc.gpsimd.collective_compute(
            "AllReduce", AluOp.add, replica_groups=rg,
            ins=[dum_in.opt()], outs=[dum_out.opt()],
        )
        warmup()
        phase1_fused01()
        nc.scalar.copy(s_sb[:], ps_ws[:])
        nc.gpsimd.dma_start(ar0_in[:], s_sb[:])
        nc.gpsimd.collective_compute(
            "AllReduce", AluOp.add, replica_groups=rg,
            ins=[ar0_in.opt()], outs=[ar0_out.opt()],
        )
        squash_v0()
        rep_v0(0)
        rep_v0(1)

        # S1: t1c0 (U1 already filled by the fused S0 pass)
        ar_c0t1 = round_(0, 1)
        # S2: t1c1
        ar_c1t1 = round_(1, 1)
        # S3: t2c0
        squash_round(0, ar_c0t1)
        ar_c0t2 = round_(0, 2)
        # S4: t2c1 (phase1 c2 overlaps; U0 free after t2c0)
        squash_round(1, ar_c1t1)
        rep_v0(2)
        phase1(2, 0, 16)
        agreement(1, 2)
        softmax(1, 2)
        phase1(2, 16, NG)
        ar_c1t2 = weighted_sum(1, 2)
        # S5: t1c2 (phase1 c3 overlaps; U1 free after t2c1)
        squash_final(0, ar_c0t2)
        rep_v0(3)
        phase1(3, 0, 16)
        agreement(2, 1)
        softmax(2, 1)
        phase1(3, 16, NG)
        ar_c2t1 = weighted_sum(2, 1)
        # S6: t1c3
        squash_final(1, ar_c1t2)
        ar_c3t1 = round_(3, 1)
        # S7: t2c2
        squash_round(2, ar_c2t1)
        ar_c2t2 = round_(2, 2)
        # S8: t2c3
        squash_round(3, ar_c3t1)
        ar_c3t2 = round_(3, 2)
        # tail
        squash_final(2, ar_c2t2)
        squash_final(3, ar_c3t2)

    _fix_sync_waits(nc)
    return nc


# ---------------------------------------------------------------- host prep
def _prep_inputs(x, W):
    """Per-core input maps.

    Local capsule l = g*8 + e (g in [0,32), e in [0,8)).
    SBUF rows r128 = e*16 + k.
      wt[(e,k); g*1024 + d*32 + j]         = W[l(g,e), j, d, k]
      xin[(e',k); c*4096 + g*128 + e*16+bb] = [e==e'] x[c*16+bb, l(g,e), k]
      xd[(e,k); g*64 + b]                  = x[b, l(g,e), k] / 32
    """
    import jax.numpy as jnp

    def tobf(a):
        return np.asarray(jnp.asarray(a).astype(jnp.bfloat16))

    in_maps = []
    ident = tobf(np.eye(128, dtype=np.float32))
    ones16 = np.zeros((128, BC), np.float32)
    for p in range(128):
        ones16[p, p % BC] = 1.0
    ones16 = tobf(ones16)
    for c in range(N_CORES):
        xi = np.asarray(x[:, c * I_LOC:(c + 1) * I_LOC, :])   # [B, 256, 16]
        wi = np.asarray(W[0, c * I_LOC:(c + 1) * I_LOC])      # [256, 32, 32, 16]

        w5 = wi.reshape(NG, 8, N_OUT, D_OUT, D_IN)            # g,e,j,d,k
        wt = np.transpose(w5, (1, 4, 0, 3, 2)).reshape(128, UCOLS)

        x5 = xi.reshape(NCHUNK, BC, NG, 8, D_IN)              # c,bb,g,e,k
        xin = np.zeros((8, D_IN, NCHUNK, NG, 8, BC), np.float32)
        for e in range(8):
            xin[e, :, :, :, e, :] = np.transpose(
                x5[:, :, :, e, :], (3, 0, 2, 1))
        xin = xin.reshape(128, NCHUNK * NG * 128)

        xd = (np.transpose(xi.reshape(B, NG, 8, D_IN),
                           (2, 3, 1, 0)) / 32.0).reshape(128, NG * B)

        in_maps.append({
            "wt": tobf(np.ascontiguousarray(wt)),
            "xin": tobf(np.ascontiguousarray(xin)),
            "xd": tobf(np.ascontiguousarray(xd)),
            "ident": ident,
            "ones16": ones16,
        })
    return in_maps


_cached = {}


def _get_program():
    if "nc" not in _cached:
        _cached["nc"] = _build_program()
    return _cached["nc"]


def kernel(x, W):
    x = np.asarray(x, dtype=np.float32)
    W = np.asarray(W, dtype=np.float32)
    nc = _get_program()
    in_maps = _prep_inputs(x, W)
    res = bass_utils.run_bass_kernel_spmd(
        nc, in_maps, core_ids=list(range(N_CORES))
    )
    v = res.results[0]["v"].reshape(B, N_OUT, D_OUT)
    return v.astype(np.float32)


# revision 17
# speedup vs baseline: 1.0179x; 1.0179x over previous
"""Trainium2 Bass kernel for CapsuleLayer dynamic routing (v5).

Problem: x [64, 2048, 16], W [1, 2048, 32, 32, 16] ->
  u_hat = einsum('bik,ijdk->bijd', x, W[0])           [B, N_in, N_out, D_out]
  3 rounds of routing (softmax over j, weighted sum over i, squash),
  returns v [64, 32, 32].

Sharding: N_in (2048) split over 8 cores, 256 local capsules each; per-round
partial weighted sums AllReduced; softmax/squash replicated.

v5 redesign (vs the v2 baseline at 745us):
  * Batch in 4 chunks of 16; chunk round-chains are interleaved two at a
    time so every AllReduce's latency hides under the other chunk's DVE
    work (U double-buffered at 64KB/partition per chunk).
  * phase 1 emits u_hat with M=128 PSUM rows ((e8,b16) out partitions,
    K=(e8,k16)=128 zero-interleaved x stationary, W streamed as the
    moving operand): 4x less PE time than the 32-row-tile baseline.
  * The agreement's reduce over d is NOT a DVE tree: the PE accumulates
    the 32 d-slices of the product P=U*v into logits PSUM through an
    identity stationary (f32 accumulation, frees ~130us of DVE).
  * Weighted-sum keeps the ones-matmul i-reduction; products (U*v, U*c)
    are the only big DVE work left, at 2x_1p bf16.
  * All PSUM->SBUF drains on ACT; AllReduce machinery + replicate DMAs
    on the Pool queue (interleaved in dependency order); W streamed once
    per chunk on the sync queue.
"""
import sys

sys.path.insert(0, '/opt/trn_rl_repo')

import numpy as np

import concourse.bass as bass
import concourse.mybir as mybir
from concourse import bass_utils, tile

# ---------------------------------------------------------------- constants
N_CORES = 8
B = 64
N_IN = 2048
D_IN = 16
N_OUT = 32
D_OUT = 32
EPS = 1e-9

I_LOC = N_IN // N_CORES          # 256 local capsules
NG = 32                          # capsule groups of 8 (i = g*8 + e)
BC = 16                          # batch chunk
NCHUNK = B // BC                 # 4
JD = N_OUT * D_OUT               # 1024 (d,j) values per capsule
UCOLS = NG * JD                  # 32768 U columns per chunk
SECG = 4                         # capsule groups per section
NSEC = NG // SECG                # 8 sections per chunk
SEC_COLS = SECG * JD             # 4096

f32 = mybir.dt.float32
bf16 = mybir.dt.bfloat16

_MAX_WAITS = 1
_carrier = [0]


def _patch_tile():
    """Work around this walrus build rejecting >1 sync wait per instruction."""
    import concourse.mybir as _mybir
    from concourse import tile as _tile
    from concourse.tile import TileContext as _TC

    def _drain_and_barrier(self, tick_clock, wait_clock):
        ScopedClock = _tile.ScopedClock
        probe = self.nc.sync.nop(nofuse=True)
        wait_clock.add_sem_waits(
            probe.ins, ScopedClock({None: tick_clock.global_clock})
        )
        si = probe.ins.sync_info
        waits = list(si.on_wait)
        probe.ins.sync_info = _mybir.SyncInfo(
            on_wait=waits[:1], on_update=list(si.on_update)
        )
        for w in waits[1:]:
            carrier = self.nc.sync.nop(nofuse=True)
            carrier.ins.sync_info = _mybir.SyncInfo(on_wait=[w], on_update=[])
        self.nc.sync.drain()
        self.nc.all_engine_barrier()
        assert self.sems is not None
        popped = self.nc._tile_sem_poison_stack.pop()
        assert popped is self._sem_poison
        self.nc.clear_and_free_semaphores(list(self.sems.allocated().values()))
        self.nc.all_engine_barrier()

    _TC._drain_and_barrier = _drain_and_barrier

    try:
        from concourse import tile_utils
        tile_utils.max_sbuf_usage = 208 * 1024
    except Exception:
        pass


def _fix_sync_waits(nc, max_waits=_MAX_WAITS):
    n_fixed = 0
    for func in nc.m.functions:
        for bb in func.blocks:
            insts = list(bb.instructions)
            new_list = []
            changed = False
            for inst in insts:
                si = getattr(inst, "sync_info", None)
                waits = list(si.on_wait) if si is not None else []
                if len(waits) > max_waits:
                    keep = waits[: max_waits - 1] if max_waits > 1 else []
                    hoist = waits[len(keep):-1]
                    tail = [waits[-1]]
                    for w in hoist:
                        _carrier[0] += 1
                        nop = mybir.InstNoOp(
                            name=f"syncfix-{_carrier[0]}", engine=inst.engine
                        )
                        nop.sync_info = mybir.SyncInfo(on_wait=[w], on_update=[])
                        new_list.append(nop)
                    inst.sync_info = mybir.SyncInfo(
                        on_wait=keep + tail, on_update=list(si.on_update)
                    )
                    changed = True
                    n_fixed += 1
                new_list.append(inst)
            if changed:
                bb.instructions = new_list
    return n_fixed


# ---------------------------------------------------------------- program
def _build_program():
    _patch_tile()
    nc = bass.Bass(trn_type="TRN2", num_devices=N_CORES)

    wt_in = nc.dram_tensor("wt", [128, UCOLS], bf16, kind="ExternalInput")
    xin_in = nc.dram_tensor("xin", [128, NCHUNK * NG * 128], bf16,
                            kind="ExternalInput")
    xd_in = nc.dram_tensor("xd", [128, NG * B], bf16, kind="ExternalInput")
    id_in = nc.dram_tensor("ident", [128, 128], bf16, kind="ExternalInput")
    ones_in = nc.dram_tensor("ones16", [128, BC], bf16, kind="ExternalInput")
    v_out = nc.dram_tensor("v", [B, JD], f32, kind="ExternalOutput")

    AluOp = mybir.AluOpType
    Act = mybir.ActivationFunctionType
    Axis = mybir.AxisListType
    rg = [list(range(N_CORES))]

    _widx = [0]
    from contextlib import ExitStack
    with tile.TileContext(nc, num_cores=N_CORES) as tc, ExitStack() as es:
        cpool = es.enter_context(tc.tile_pool(name="const", bufs=1))
        wpool = es.enter_context(tc.tile_pool(name="wstream", bufs=4))
        upool = es.enter_context(tc.tile_pool(name="ubuf", bufs=1))
        scpool = es.enter_context(tc.tile_pool(name="scratch", bufs=1))
        smpool = es.enter_context(tc.tile_pool(name="small", bufs=1))
        psph = es.enter_context(tc.tile_pool(name="psph1", bufs=2, space="PSUM"))
        pslg = es.enter_context(tc.tile_pool(name="pslog", bufs=1, space="PSUM"))
        psws = es.enter_context(tc.tile_pool(name="psws", bufs=1, space="PSUM"))
        dpool = es.enter_context(tc.tile_pool(name="dram", bufs=1, space="DRAM"))

        # ---- constants / inputs resident in SBUF
        ident = cpool.tile([128, 128], bf16, tag="ident")
        ones16 = cpool.tile([128, BC], bf16, tag="ones16")
        xd = cpool.tile([128, NG * B], bf16, tag="xd")
        nc.sync.dma_start(ident[:], id_in[:])
        nc.sync.dma_start(ones16[:], ones_in[:])
        nc.sync.dma_start(xd[:], xd_in[:])

        # ---- big buffers
        U = [upool.tile([128, UCOLS], bf16, tag=f"U{h}", name=f"U{h}")
             for h in range(2)]
        xint = [cpool.tile([128, NG * 128], bf16, tag=f"xint{h}",
                           name=f"xint{h}") for h in range(2)]
        pbuf = scpool.tile([128, 2 * SEC_COLS], bf16, tag="pbuf")
        slots = [pbuf[:, :SEC_COLS], pbuf[:, SEC_COLS:]]
        algA = [smpool.tile([128, NG * N_OUT], f32, tag=f"algA{h}",
                            name=f"algA{h}") for h in range(2)]
        cbuf = [smpool.tile([128, NG * N_OUT], bf16, tag=f"c{h}",
                            name=f"c{h}") for h in range(2)]
        vt = [smpool.tile([128, JD], bf16, tag=f"vt{h}", name=f"vt{h}")
              for h in range(2)]
        ebuf = smpool.tile([128, NG * N_OUT], f32, tag="ebuf")
        s_rep = smpool.tile([128, JD], f32, tag="srep")
        v0all = smpool.tile([128, JD], bf16, tag="v0all")
        Zt = smpool.tile([128, NG], f32, tag="Zt")
        Zr = smpool.tile([128, NG], f32, tag="Zr")
        # aliases: squash's square scratch reuses ebuf; the softmax
        # Zr-replica reuses s_rep (lifetimes strictly serialized on DVE).
        p2 = ebuf
        zrep = s_rep
        s2 = smpool.tile([128, N_OUT], f32, tag="s2")
        lns = smpool.tile([128, N_OUT], f32, tag="lns")
        rsq = smpool.tile([128, N_OUT], f32, tag="rsq")
        den = smpool.tile([128, N_OUT], f32, tag="den")
        rinv = smpool.tile([128, N_OUT], f32, tag="rinv")
        fsc = smpool.tile([128, N_OUT], f32, tag="fsc")
        ibuf = smpool.tile([128, N_OUT], mybir.dt.int32, tag="ibuf")
        s_sb = smpool.tile([B, JD], f32, tag="ssb")
        v_fin = smpool.tile([BC, JD], f32, tag="vfin")

        # PSUM tiles
        ps_log = pslg.tile([128, NG * N_OUT], f32, tag="pslog")
        ps_ws = psws.tile([B, JD], f32, tag="psws")

        # AR dram staging
        dum_in = dpool.tile([16, 16], f32, tag="dumi")
        dum_out = dpool.tile([16, 16], f32, tag="dumo")
        ar0_in = dpool.tile([B, JD], f32, tag="ar0i")
        ar0_out = dpool.tile([B, JD], f32, tag="ar0o")
        ar_bufs = {}
        for q in range(NCHUNK):
            for t in (1, 2):
                ar_bufs[(q, t)] = (
                    dpool.tile([BC, JD], f32, name=f"ari{q}{t}"),
                    dpool.tile([BC, JD], f32, name=f"aro{q}{t}"),
                )

        # ------------------------------------------------------ phase 1
        def warmup():
            """Back-to-back full matmuls to ramp the PE p-state before the
            s0/phase-1 burst (output never read)."""
            ps = psph.tile([128, JD], f32, tag="ph1", name="warm")
            for i in range(16):
                nc.tensor.matmul(
                    ps[:, 0:512], xd[:, 0:128], xd[:, 0:512],
                    start=True, stop=True,
                )

        def phase1_fused01():
            """One W pass serving s0 (full batch) + phase1 of chunks 0 and
            1: per W tile, 2 s0-matmuls accumulate into ps_ws and 2+2
            matmuls fill U0/U1; c0 PSUMs drain on DVE (idle during S0),
            c1 on ACT."""
            nc.sync.dma_start(xint[0][:], xin_in[:, 0:NG * 128])
            nc.sync.dma_start(
                xint[1][:], xin_in[:, NG * 128:2 * NG * 128])
            for g in range(NG):
                w = wpool.tile([128, JD], bf16, tag="w")
                dmaq = nc.sync if g % 2 == 0 else nc.scalar
                dmaq.dma_start(w[:], wt_in[:, g * JD:(g + 1) * JD])
                psA = psph.tile([128, JD], f32, tag="ph1", name=f"psA{g}")
                psB = psph.tile([128, JD], f32, tag="ph1", name=f"psB{g}")
                for half in range(2):
                    nc.tensor.matmul(
                        psA[:, half * 512:(half + 1) * 512],
                        xint[0][:, g * 128:(g + 1) * 128],
                        w[:, half * 512:(half + 1) * 512],
                        start=True, stop=True,
                    )
                    nc.tensor.matmul(
                        psB[:, half * 512:(half + 1) * 512],
                        xint[1][:, g * 128:(g + 1) * 128],
                        w[:, half * 512:(half + 1) * 512],
                        start=True, stop=True,
                    )
                nc.vector.tensor_copy(U[0][:, g * JD:(g + 1) * JD], psA[:])
                nc.scalar.copy(U[1][:, g * JD:(g + 1) * JD], psB[:])
                for half in range(2):
                    nc.tensor.matmul(
                        ps_ws[:, half * 512:(half + 1) * 512],
                        xd[:, g * B:(g + 1) * B],
                        w[:, half * 512:(half + 1) * 512],
                        start=(g == 0), stop=(g == NG - 1),
                    )

        def phase1(q, g0=0, g1=NG, with_s0=False):
            """u_hat groups [g0,g1) for batch chunk q into U[q%2]; W
            streamed per group on the sync queue; PSUM drained on ACT.
            If with_s0, also accumulates s0 = sum_i u_hat/32 for the full
            batch into ps_ws through the dense xd stationary.  Mid-kernel
            chunks are emitted in two halves around the concurrent round's
            softmax so the ACT/PE FIFOs never head-of-line-block it."""
            Uq = U[q % 2]
            xq = xint[q % 2]
            if g0 == 0:
                nc.sync.dma_start(
                    xq[:], xin_in[:, q * NG * 128:(q + 1) * NG * 128])
            for g in range(g0, g1):
                w = wpool.tile([128, JD], bf16, tag="w")
                dmaq = nc.sync if g % 2 == 0 else nc.gpsimd
                dmaq.dma_start(w[:], wt_in[:, g * JD:(g + 1) * JD])
                ps = psph.tile([128, JD], f32, tag="ph1")
                for half in range(2):
                    if with_s0:
                        nc.tensor.matmul(
                            ps_ws[:, half * 512:(half + 1) * 512],
                            xd[:, g * B:(g + 1) * B],
                            w[:, half * 512:(half + 1) * 512],
                            start=(g == 0), stop=(g == NG - 1),
                        )
                    nc.tensor.matmul(
                        ps[:, half * 512:(half + 1) * 512],
                        xq[:, g * 128:(g + 1) * 128],
                        w[:, half * 512:(half + 1) * 512],
                        start=True, stop=True,
                    )
                nc.scalar.copy(Uq[:, g * JD:(g + 1) * JD], ps[:])

        def pe_warm(n):
            """Dependency-free filler matmuls that hold the PE p-state at
            full clock across short DVE-product waits."""
            ps = psph.tile([128, JD], f32, tag="ph1", name=f"warmf{_widx[0]}")
            _widx[0] += 1
            for i in range(n):
                nc.tensor.matmul(
                    ps[:, 0:128], ident[:], ident[:],
                    start=True, stop=True,
                )

        # ------------------------------------------------------ routing ops
        def agreement(q, t, warm=True):
            """logits psum[p=(e,b), (g,j)] = sum_d U*v via DVE product +
            PE identity-matmul accumulation over the 32 d slices."""
            if warm:
                pe_warm(20)
            Uq = U[q % 2]
            v4 = (vt[q % 2][:]
                  .rearrange("p (d j) -> p d j", d=D_OUT, j=N_OUT)
                  .unsqueeze(1)
                  .to_broadcast((128, SECG, D_OUT, N_OUT)))
            P2 = pbuf[:].rearrange(
                "p (g d j) -> p g d j", g=2 * SECG, d=D_OUT, j=N_OUT)
            for sec in range(NSEC):
                slot = slots[sec % 2]
                P = slot.rearrange(
                    "p (g d j) -> p g d j", g=SECG, d=D_OUT, j=N_OUT)
                Us = Uq[:, sec * SEC_COLS:(sec + 1) * SEC_COLS].rearrange(
                    "p (g d j) -> p g d j", g=SECG, d=D_OUT, j=N_OUT)
                nc.vector.tensor_tensor(P, Us, v4, AluOp.mult)   # 2x
                if sec % 2 == 1:
                    # one identity-matmul sweep reduces BOTH sections of
                    # the pair buffer (N=256: halves the LDWEIGHTS tax)
                    for dd in range(D_OUT):
                        nc.tensor.matmul(
                            ps_log[:, (sec - 1) * SECG * N_OUT:
                                   (sec + 1) * SECG * N_OUT],
                            ident[:],
                            P2[:, :, dd, :],
                            start=(dd == 0), stop=(dd == D_OUT - 1),
                        )
                    if warm and sec < NSEC - 1:
                        pe_warm(5)

        def softmax(q, t):
            """c = softmax over j of logits (+ prev-round logits for t=2)."""
            A = algA[q % 2]
            if t == 1:
                nc.scalar.copy(A[:], ps_log[:])
            else:
                nc.vector.tensor_add(A[:], A[:], ps_log[:])
            nc.scalar.activation(ebuf[:], A[:], Act.Exp)
            e3 = ebuf[:].rearrange("p (g j) -> p g j", g=NG, j=N_OUT)
            nc.vector.reduce_sum(Zt[:], e3, axis=Axis.X)
            nc.vector.reciprocal(Zr[:], Zt[:])
            nc.vector.tensor_copy(
                zrep[:].rearrange("p (g j) -> p g j", g=NG, j=N_OUT),
                Zr[:].unsqueeze(2).to_broadcast((128, NG, N_OUT)))
            nc.vector.tensor_tensor(
                cbuf[q % 2][:], ebuf[:], zrep[:], AluOp.mult)

        def weighted_sum(q, t, warm=True):
            """s_partial[b,(d,j)] = sum_i c*U: DVE product (2x) + PE
            ones-matmul reduction over (e-partitions, g-psum-accum)."""
            if warm:
                pe_warm(20)
            Uq = U[q % 2]
            cq = cbuf[q % 2]
            for sec in range(NSEC):
                slot = slots[sec % 2]
                P = slot[:].rearrange(
                    "p (g d j) -> p g d j", g=SECG, d=D_OUT, j=N_OUT)
                Us = Uq[:, sec * SEC_COLS:(sec + 1) * SEC_COLS].rearrange(
                    "p (g d j) -> p g d j", g=SECG, d=D_OUT, j=N_OUT)
                c4 = (cq[:, sec * SECG * N_OUT:(sec + 1) * SECG * N_OUT]
                      .rearrange("p (g j) -> p g j", g=SECG, j=N_OUT)
                      .unsqueeze(2)
                      .to_broadcast((128, SECG, D_OUT, N_OUT)))
                nc.vector.tensor_tensor(P, Us, c4, AluOp.mult)   # 2x
                for g in range(SECG):
                    for half in range(2):
                        nc.tensor.matmul(
                            ps_ws[0:BC, half * 512:(half + 1) * 512],
                            ones16[:],
                            slot[:, g * JD + half * 512:
                                 g * JD + half * 512 + 512],
                            start=(sec == 0 and g == 0),
                            stop=(sec == NSEC - 1 and g == SECG - 1),
                        )
            nc.scalar.copy(s_sb[0:BC, :], ps_ws[0:BC, :])
            ar_in, ar_out = ar_bufs[(q, t)]
            nc.gpsimd.dma_start(ar_in[:], s_sb[0:BC, :])
            nc.gpsimd.collective_compute(
                "AllReduce", AluOp.add, replica_groups=rg,
                ins=[ar_in.opt()], outs=[ar_out.opt()],
            )
            return ar_out

        def squash_core(rows, out_tt):
            """Common squash tail: rows = partition count holding s in
            s_rep; out_tt(s3, f3) emits the final multiply."""
            nc.scalar.square(p2[0:rows, :], s_rep[0:rows, :])
            p3 = p2[0:rows, :].rearrange("p (d j) -> p j d", d=D_OUT, j=N_OUT)
            nc.vector.reduce_sum(s2[0:rows, :], p3, axis=Axis.X)
            nc.vector.tensor_scalar_add(den[0:rows, :], s2[0:rows, :],
                                        1.0 + EPS)
            nc.vector.tensor_scalar_add(lns[0:rows, :], s2[0:rows, :], EPS)
            ii = lns[0:rows, :].bitcast(mybir.dt.int32)
            nc.vector.tensor_scalar(
                ibuf[0:rows, :], ii, 1, None,
                mybir.AluOpType.logical_shift_right)
            nc.vector.tensor_scalar(
                ibuf[0:rows, :], ibuf[0:rows, :], 0x5F3759DF, -1,
                mybir.AluOpType.subtract, mybir.AluOpType.mult)
            y0 = ibuf[0:rows, :].bitcast(f32)
            nc.vector.tensor_mul(rsq[0:rows, :], y0, y0)
            nc.vector.tensor_mul(rsq[0:rows, :], rsq[0:rows, :],
                                 lns[0:rows, :])
            nc.vector.tensor_scalar(
                rsq[0:rows, :], rsq[0:rows, :], -0.5, 1.5,
                mybir.AluOpType.mult, mybir.AluOpType.add)
            nc.vector.tensor_mul(rsq[0:rows, :], rsq[0:rows, :], y0)
            nc.vector.tensor_mul(fsc[0:rows, :], rsq[0:rows, :],
                                 rsq[0:rows, :])
            nc.vector.tensor_mul(fsc[0:rows, :], fsc[0:rows, :],
                                 lns[0:rows, :])
            nc.vector.tensor_scalar(
                fsc[0:rows, :], fsc[0:rows, :], -0.5, 1.5,
                mybir.AluOpType.mult, mybir.AluOpType.add)
            nc.vector.tensor_mul(rsq[0:rows, :], rsq[0:rows, :],
                                 fsc[0:rows, :])
            nc.vector.reciprocal(rinv[0:rows, :], den[0:rows, :])
            nc.vector.tensor_mul(fsc[0:rows, :], rsq[0:rows, :],
                                 rinv[0:rows, :])
            nc.vector.tensor_mul(fsc[0:rows, :], fsc[0:rows, :],
                                 s2[0:rows, :])
            out_tt()

        def squash_v0():
            """v0 = squash(AllReduce(s0)) for the full batch at
            [p=(rep2,b64)]."""
            for r in range(2):
                nc.gpsimd.dma_start(
                    s_rep[64 * r:64 * r + 64, :], ar0_out[:])

            def tt():
                s3 = s_rep[:].rearrange("p (d j) -> p d j", d=D_OUT, j=N_OUT)
                f3 = fsc[:].unsqueeze(1).to_broadcast((128, D_OUT, N_OUT))
                v3 = v0all[:].rearrange("p (d j) -> p d j", d=D_OUT, j=N_OUT)
                nc.vector.tensor_tensor(v3, s3, f3, AluOp.mult)
            squash_core(128, tt)

        def rep_v0(q):
            """vt[q%2] <- per-chunk (e8,b16)-replicated slice of v0all."""
            dst = vt[q % 2]
            for e in range(8):
                nc.gpsimd.dma_start(
                    dst[16 * e:16 * e + 16, :],
                    v0all[q * BC:(q + 1) * BC, :])

        def rep_s(ar_out):
            """Replicate an AllReduced [16,1024] s into s_rep's 8 e-groups
            (Pool queue; emit right after the producing collective so the
            queue never head-blocks)."""
            for e in range(8):
                nc.gpsimd.dma_start(
                    s_rep[16 * e:16 * e + 16, :], ar_out[:])

        def squash_round(q, ar_out, reps_done=False):
            """v_{t} for chunk q from its AllReduced s, into vt[q%2]."""
            if not reps_done:
                rep_s(ar_out)

            def tt():
                s3 = s_rep[:].rearrange("p (d j) -> p d j", d=D_OUT, j=N_OUT)
                f3 = fsc[:].unsqueeze(1).to_broadcast((128, D_OUT, N_OUT))
                v3 = vt[q % 2][:].rearrange(
                    "p (d j) -> p d j", d=D_OUT, j=N_OUT)
                nc.vector.tensor_tensor(v3, s3, f3, AluOp.mult)
            squash_core(128, tt)

        def squash_final(q, ar_out):
            """Final v for chunk q -> v_out rows, reference layout."""
            for e in range(8):
                nc.gpsimd.dma_start(
                    s_rep[16 * e:16 * e + 16, :], ar_out[:])

            def tt():
                vf = v_fin[:].rearrange("p (j d) -> p d j", j=N_OUT, d=D_OUT)
                nc.vector.tensor_tensor(
                    vf,
                    s_rep[0:BC, :].rearrange(
                        "p (d j) -> p d j", d=D_OUT, j=N_OUT),
                    fsc[0:BC, :].unsqueeze(1).to_broadcast(
                        (BC, D_OUT, N_OUT)),
                    AluOp.mult)
                nc.gpsimd.dma_start(v_out[q * BC:(q + 1) * BC, :], v_fin[:])
            squash_core(BC, tt)

        def round_(q, t):
            agreement(q, t)
            softmax(q, t)
            return weighted_sum(q, t)

        # ------------------------------------------------------ emission
        # S0: warm PE, stream W once for chunk 0 while accumulating s0 for
        # the full batch; AllReduce s0; squash v0.
        # CC cold-start warmup: a tiny AllReduce enqueued at t=0 absorbs
        # the ~40us first-collective spin-up while S0 computes.
        nc.gpsimd.dma_start(dum_in[:], s_sb[0:16, 0:16])
        nc.gpsimd.collective_compute(
            "AllReduce", AluOp.add, replica_groups=rg,
            ins=[dum_in.opt()], outs=[dum_out.opt()],
        )
        warmup()
        phase1_fused01()
        nc.scalar.copy(s_sb[:], ps_ws[:])
        nc.gpsimd.dma_start(ar0_in[:], s_sb[:])
        nc.gpsimd.collective_compute(
            "AllReduce", AluOp.add, replica_groups=rg,
            ins=[ar0_in.opt()], outs=[ar0_out.opt()],
        )
        squash_v0()
        rep_v0(0)
        rep_v0(1)

        # S1: t1c0 (U1 already filled by the fused S0 pass)
        ar_c0t1 = round_(0, 1)
        # S2: t1c1
        ar_c1t1 = round_(1, 1)
        # S3: t2c0
        squash_round(0, ar_c0t1)
        ar_c0t2 = round_(0, 2)
        # S4: t2c1 (phase1 c2 overlaps; U0 free after t2c0)
        squash_round(1, ar_c1t1)
        rep_v0(2)
        phase1(2, 0, 16)
        agreement(1, 2, warm=False)
        softmax(1, 2)
        phase1(2, 16, NG)
        ar_c1t2 = weighted_sum(1, 2, warm=False)
        # S5: t1c2 (phase1 c3 overlaps; U1 free after t2c1)
        squash_final(0, ar_c0t2)
        rep_v0(3)
        phase1(3, 0, 16)
        agreement(2, 1, warm=False)
        softmax(2, 1)
        phase1(3, 16, NG)
        ar_c2t1 = weighted_sum(2, 1, warm=False)
        # S6: t1c3
        squash_final(1, ar_c1t2)
        ar_c3t1 = round_(3, 1)
        # S7: t2c2
        squash_round(2, ar_c2t1)
        ar_c2t2 = round_(2, 2)
        # S8: t2c3
        squash_round(3, ar_c3t1)
        ar_c3t2 = round_(3, 2)
        # tail
        squash_final(2, ar_c2t2)
        squash_final(3, ar_c3t2)

    _fix_sync_waits(nc)
    return nc


# ---------------------------------------------------------------- host prep
def _prep_inputs(x, W):
    """Per-core input maps.

    Local capsule l = g*8 + e (g in [0,32), e in [0,8)).
    SBUF rows r128 = e*16 + k.
      wt[(e,k); g*1024 + d*32 + j]         = W[l(g,e), j, d, k]
      xin[(e',k); c*4096 + g*128 + e*16+bb] = [e==e'] x[c*16+bb, l(g,e), k]
      xd[(e,k); g*64 + b]                  = x[b, l(g,e), k] / 32
    """
    import jax.numpy as jnp

    def tobf(a):
        return np.asarray(jnp.asarray(a).astype(jnp.bfloat16))

    in_maps = []
    ident = tobf(np.eye(128, dtype=np.float32))
    ones16 = np.zeros((128, BC), np.float32)
    for p in range(128):
        ones16[p, p % BC] = 1.0
    ones16 = tobf(ones16)
    for c in range(N_CORES):
        xi = np.asarray(x[:, c * I_LOC:(c + 1) * I_LOC, :])   # [B, 256, 16]
        wi = np.asarray(W[0, c * I_LOC:(c + 1) * I_LOC])      # [256, 32, 32, 16]

        w5 = wi.reshape(NG, 8, N_OUT, D_OUT, D_IN)            # g,e,j,d,k
        wt = np.transpose(w5, (1, 4, 0, 3, 2)).reshape(128, UCOLS)

        x5 = xi.reshape(NCHUNK, BC, NG, 8, D_IN)              # c,bb,g,e,k
        xin = np.zeros((8, D_IN, NCHUNK, NG, 8, BC), np.float32)
        for e in range(8):
            xin[e, :, :, :, e, :] = np.transpose(
                x5[:, :, :, e, :], (3, 0, 2, 1))
        xin = xin.reshape(128, NCHUNK * NG * 128)

        xd = (np.transpose(xi.reshape(B, NG, 8, D_IN),
                           (2, 3, 1, 0)) / 32.0).reshape(128, NG * B)

        in_maps.append({
            "wt": tobf(np.ascontiguousarray(wt)),
            "xin": tobf(np.ascontiguousarray(xin)),
            "xd": tobf(np.ascontiguousarray(xd)),
            "ident": ident,
            "ones16": ones16,
        })
    return in_maps


_cached = {}


def _get_program():
    if "nc" not in _cached:
        _cached["nc"] = _build_program()
    return _cached["nc"]


def kernel(x, W):
    x = np.asarray(x, dtype=np.float32)
    W = np.asarray(W, dtype=np.float32)
    nc = _get_program()
    in_maps = _prep_inputs(x, W)
    res = bass_utils.run_bass_kernel_spmd(
        nc, in_maps, core_ids=list(range(N_CORES))
    )
    v = res.results[0]["v"].reshape(B, N_OUT, D_OUT)
    return v.astype(np.float32)


# revision 18
# speedup vs baseline: 1.2361x; 1.2143x over previous
"""Trainium2 Bass kernel for CapsuleLayer dynamic routing (v5).

Problem: x [64, 2048, 16], W [1, 2048, 32, 32, 16] ->
  u_hat = einsum('bik,ijdk->bijd', x, W[0])           [B, N_in, N_out, D_out]
  3 rounds of routing (softmax over j, weighted sum over i, squash),
  returns v [64, 32, 32].

Sharding: N_in (2048) split over 8 cores, 256 local capsules each; per-round
partial weighted sums AllReduced; softmax/squash replicated.

v5 redesign (vs the v2 baseline at 745us):
  * Batch in 4 chunks of 16; chunk round-chains are interleaved two at a
    time so every AllReduce's latency hides under the other chunk's DVE
    work (U double-buffered at 64KB/partition per chunk).
  * phase 1 emits u_hat with M=128 PSUM rows ((e8,b16) out partitions,
    K=(e8,k16)=128 zero-interleaved x stationary, W streamed as the
    moving operand): 4x less PE time than the 32-row-tile baseline.
  * The agreement's reduce over d is NOT a DVE tree: the PE accumulates
    the 32 d-slices of the product P=U*v into logits PSUM through an
    identity stationary (f32 accumulation, frees ~130us of DVE).
  * Weighted-sum keeps the ones-matmul i-reduction; products (U*v, U*c)
    are the only big DVE work left, at 2x_1p bf16.
  * All PSUM->SBUF drains on ACT; AllReduce machinery + replicate DMAs
    on the Pool queue (interleaved in dependency order); W streamed once
    per chunk on the sync queue.
"""
import sys

sys.path.insert(0, '/opt/trn_rl_repo')

import numpy as np

import concourse.bass as bass
import concourse.mybir as mybir
from concourse import bass_utils, tile

# ---------------------------------------------------------------- constants
N_CORES = 8
B = 64
N_IN = 2048
D_IN = 16
N_OUT = 32
D_OUT = 32
EPS = 1e-9

I_LOC = N_IN // N_CORES          # 256 local capsules
NG = 32                          # capsule groups of 8 (i = g*8 + e)
BC = 16                          # batch chunk
NCHUNK = B // BC                 # 4
JD = N_OUT * D_OUT               # 1024 (d,j) values per capsule
UCOLS = NG * JD                  # 32768 U columns per chunk
SECG = 4                         # capsule groups per section
NSEC = NG // SECG                # 8 sections per chunk
SEC_COLS = SECG * JD             # 4096

f32 = mybir.dt.float32
bf16 = mybir.dt.bfloat16

_MAX_WAITS = 1
_carrier = [0]


def _patch_tile():
    """Work around this walrus build rejecting >1 sync wait per instruction."""
    import concourse.mybir as _mybir
    from concourse import tile as _tile
    from concourse.tile import TileContext as _TC

    def _drain_and_barrier(self, tick_clock, wait_clock):
        ScopedClock = _tile.ScopedClock
        probe = self.nc.sync.nop(nofuse=True)
        wait_clock.add_sem_waits(
            probe.ins, ScopedClock({None: tick_clock.global_clock})
        )
        si = probe.ins.sync_info
        waits = list(si.on_wait)
        probe.ins.sync_info = _mybir.SyncInfo(
            on_wait=waits[:1], on_update=list(si.on_update)
        )
        for w in waits[1:]:
            carrier = self.nc.sync.nop(nofuse=True)
            carrier.ins.sync_info = _mybir.SyncInfo(on_wait=[w], on_update=[])
        self.nc.sync.drain()
        self.nc.all_engine_barrier()
        assert self.sems is not None
        popped = self.nc._tile_sem_poison_stack.pop()
        assert popped is self._sem_poison
        self.nc.clear_and_free_semaphores(list(self.sems.allocated().values()))
        self.nc.all_engine_barrier()

    _TC._drain_and_barrier = _drain_and_barrier

    try:
        from concourse import tile_utils
        tile_utils.max_sbuf_usage = 208 * 1024
    except Exception:
        pass


def _fix_sync_waits(nc, max_waits=_MAX_WAITS):
    n_fixed = 0
    for func in nc.m.functions:
        for bb in func.blocks:
            insts = list(bb.instructions)
            new_list = []
            changed = False
            for inst in insts:
                si = getattr(inst, "sync_info", None)
                waits = list(si.on_wait) if si is not None else []
                if len(waits) > max_waits:
                    keep = waits[: max_waits - 1] if max_waits > 1 else []
                    hoist = waits[len(keep):-1]
                    tail = [waits[-1]]
                    for w in hoist:
                        _carrier[0] += 1
                        nop = mybir.InstNoOp(
                            name=f"syncfix-{_carrier[0]}", engine=inst.engine
                        )
                        nop.sync_info = mybir.SyncInfo(on_wait=[w], on_update=[])
                        new_list.append(nop)
                    inst.sync_info = mybir.SyncInfo(
                        on_wait=keep + tail, on_update=list(si.on_update)
                    )
                    changed = True
                    n_fixed += 1
                new_list.append(inst)
            if changed:
                bb.instructions = new_list
    return n_fixed


# ---------------------------------------------------------------- program
def _build_program():
    _patch_tile()
    nc = bass.Bass(trn_type="TRN2", num_devices=N_CORES)

    wt_in = nc.dram_tensor("wt", [128, UCOLS], bf16, kind="ExternalInput")
    xin_in = nc.dram_tensor("xin", [128, NCHUNK * NG * 128], bf16,
                            kind="ExternalInput")
    xd_in = nc.dram_tensor("xd", [128, NG * B], bf16, kind="ExternalInput")
    id_in = nc.dram_tensor("ident", [128, 128], bf16, kind="ExternalInput")
    ones_in = nc.dram_tensor("ones16", [128, BC], bf16, kind="ExternalInput")
    v_out = nc.dram_tensor("v", [B, JD], f32, kind="ExternalOutput")

    AluOp = mybir.AluOpType
    Act = mybir.ActivationFunctionType
    Axis = mybir.AxisListType
    rg = [list(range(N_CORES))]

    _widx = [0]
    from contextlib import ExitStack
    with tile.TileContext(nc, num_cores=N_CORES) as tc, ExitStack() as es:
        cpool = es.enter_context(tc.tile_pool(name="const", bufs=1))
        wpool = es.enter_context(tc.tile_pool(name="wstream", bufs=4))
        upool = es.enter_context(tc.tile_pool(name="ubuf", bufs=1))
        scpool = es.enter_context(tc.tile_pool(name="scratch", bufs=1))
        smpool = es.enter_context(tc.tile_pool(name="small", bufs=1))
        psph = es.enter_context(tc.tile_pool(name="psph1", bufs=2, space="PSUM"))
        pslg = es.enter_context(tc.tile_pool(name="pslog", bufs=1, space="PSUM"))
        psws = es.enter_context(tc.tile_pool(name="psws", bufs=1, space="PSUM"))
        dpool = es.enter_context(tc.tile_pool(name="dram", bufs=1, space="DRAM"))

        # ---- constants / inputs resident in SBUF
        ident = cpool.tile([128, 128], bf16, tag="ident")
        ones16 = cpool.tile([128, BC], bf16, tag="ones16")
        xd = cpool.tile([128, NG * B], bf16, tag="xd")
        nc.sync.dma_start(ident[:], id_in[:])
        nc.sync.dma_start(ones16[:], ones_in[:])
        nc.sync.dma_start(xd[:], xd_in[:])

        # ---- big buffers
        U = [upool.tile([128, UCOLS], bf16, tag=f"U{h}", name=f"U{h}")
             for h in range(2)]
        xint = [cpool.tile([128, NG * 128], bf16, tag=f"xint{h}",
                           name=f"xint{h}") for h in range(2)]
        slots = [scpool.tile([128, SEC_COLS], bf16, tag=f"slot{h}",
                             name=f"slot{h}") for h in range(2)]
        algA = [smpool.tile([128, NG * N_OUT], f32, tag=f"algA{h}",
                            name=f"algA{h}") for h in range(2)]
        cbuf = [smpool.tile([128, NG * N_OUT], bf16, tag=f"c{h}",
                            name=f"c{h}") for h in range(2)]
        vt = [smpool.tile([128, JD], bf16, tag=f"vt{h}", name=f"vt{h}")
              for h in range(2)]
        ebuf = smpool.tile([128, NG * N_OUT], f32, tag="ebuf")
        s_rep = smpool.tile([128, JD], f32, tag="srep")
        v0all = smpool.tile([128, JD], bf16, tag="v0all")
        Zt = smpool.tile([128, NG], f32, tag="Zt")
        Zr = smpool.tile([128, NG], f32, tag="Zr")
        # aliases: squash's square scratch reuses ebuf; the softmax
        # Zr-replica reuses s_rep (lifetimes strictly serialized on DVE).
        p2 = ebuf
        zrep = s_rep
        s2 = smpool.tile([128, N_OUT], f32, tag="s2")
        lns = smpool.tile([128, N_OUT], f32, tag="lns")
        rsq = smpool.tile([128, N_OUT], f32, tag="rsq")
        den = smpool.tile([128, N_OUT], f32, tag="den")
        rinv = smpool.tile([128, N_OUT], f32, tag="rinv")
        fsc = smpool.tile([128, N_OUT], f32, tag="fsc")
        ibuf = smpool.tile([128, N_OUT], mybir.dt.int32, tag="ibuf")
        s_sb = smpool.tile([B, JD], f32, tag="ssb")
        v_fin = smpool.tile([BC, JD], f32, tag="vfin")

        # PSUM tiles
        ps_log = pslg.tile([128, NG * N_OUT], f32, tag="pslog")
        ps_ws = psws.tile([B, JD], f32, tag="psws")

        # AR dram staging
        dum_in = dpool.tile([16, 16], f32, tag="dumi")
        dum_out = dpool.tile([16, 16], f32, tag="dumo")
        ar0_in = dpool.tile([B, JD], f32, tag="ar0i")
        ar0_out = dpool.tile([B, JD], f32, tag="ar0o")
        ar_bufs = {}
        for q in range(NCHUNK):
            for t in (1, 2):
                ar_bufs[(q, t)] = (
                    dpool.tile([BC, JD], f32, name=f"ari{q}{t}"),
                    dpool.tile([BC, JD], f32, name=f"aro{q}{t}"),
                )

        # ------------------------------------------------------ phase 1
        def warmup():
            """Back-to-back full matmuls to ramp the PE p-state before the
            s0/phase-1 burst (output never read)."""
            ps = psph.tile([128, JD], f32, tag="ph1", name="warm")
            for i in range(16):
                nc.tensor.matmul(
                    ps[:, 0:512], xd[:, 0:128], xd[:, 0:512],
                    start=True, stop=True,
                )

        def phase1_fused01():
            """One W pass serving s0 (full batch) + phase1 of chunks 0 and
            1: per W tile, 2 s0-matmuls accumulate into ps_ws and 2+2
            matmuls fill U0/U1; c0 PSUMs drain on DVE (idle during S0),
            c1 on ACT."""
            nc.sync.dma_start(xint[0][:], xin_in[:, 0:NG * 128])
            nc.sync.dma_start(
                xint[1][:], xin_in[:, NG * 128:2 * NG * 128])
            for g in range(NG):
                w = wpool.tile([128, JD], bf16, tag="w")
                dmaq = nc.sync if g % 2 == 0 else nc.scalar
                dmaq.dma_start(w[:], wt_in[:, g * JD:(g + 1) * JD])
                psA = psph.tile([128, JD], f32, tag="ph1", name=f"psA{g}")
                psB = psph.tile([128, JD], f32, tag="ph1", name=f"psB{g}")
                for half in range(2):
                    nc.tensor.matmul(
                        psA[:, half * 512:(half + 1) * 512],
                        xint[0][:, g * 128:(g + 1) * 128],
                        w[:, half * 512:(half + 1) * 512],
                        start=True, stop=True,
                    )
                    nc.tensor.matmul(
                        psB[:, half * 512:(half + 1) * 512],
                        xint[1][:, g * 128:(g + 1) * 128],
                        w[:, half * 512:(half + 1) * 512],
                        start=True, stop=True,
                    )
                nc.vector.tensor_copy(U[0][:, g * JD:(g + 1) * JD], psA[:])
                nc.scalar.copy(U[1][:, g * JD:(g + 1) * JD], psB[:])
                for half in range(2):
                    nc.tensor.matmul(
                        ps_ws[:, half * 512:(half + 1) * 512],
                        xd[:, g * B:(g + 1) * B],
                        w[:, half * 512:(half + 1) * 512],
                        start=(g == 0), stop=(g == NG - 1),
                    )

        def phase1(q, g0=0, g1=NG, with_s0=False):
            """u_hat groups [g0,g1) for batch chunk q into U[q%2]; W
            streamed per group on the sync queue; PSUM drained on ACT.
            If with_s0, also accumulates s0 = sum_i u_hat/32 for the full
            batch into ps_ws through the dense xd stationary.  Mid-kernel
            chunks are emitted in two halves around the concurrent round's
            softmax so the ACT/PE FIFOs never head-of-line-block it."""
            Uq = U[q % 2]
            xq = xint[q % 2]
            if g0 == 0:
                nc.sync.dma_start(
                    xq[:], xin_in[:, q * NG * 128:(q + 1) * NG * 128])
            for g in range(g0, g1):
                w = wpool.tile([128, JD], bf16, tag="w")
                dmaq = nc.sync if g % 2 == 0 else nc.gpsimd
                dmaq.dma_start(w[:], wt_in[:, g * JD:(g + 1) * JD])
                ps = psph.tile([128, JD], f32, tag="ph1")
                for half in range(2):
                    if with_s0:
                        nc.tensor.matmul(
                            ps_ws[:, half * 512:(half + 1) * 512],
                            xd[:, g * B:(g + 1) * B],
                            w[:, half * 512:(half + 1) * 512],
                            start=(g == 0), stop=(g == NG - 1),
                        )
                    nc.tensor.matmul(
                        ps[:, half * 512:(half + 1) * 512],
                        xq[:, g * 128:(g + 1) * 128],
                        w[:, half * 512:(half + 1) * 512],
                        start=True, stop=True,
                    )
                nc.scalar.copy(Uq[:, g * JD:(g + 1) * JD], ps[:])

        def pe_warm(n):
            """Dependency-free filler matmuls that hold the PE p-state at
            full clock across short DVE-product waits."""
            ps = psph.tile([128, JD], f32, tag="ph1", name=f"warmf{_widx[0]}")
            _widx[0] += 1
            for i in range(n):
                nc.tensor.matmul(
                    ps[:, 0:128], ident[:], ident[:],
                    start=True, stop=True,
                )

        # ------------------------------------------------------ routing ops
        def agreement(q, t, warm=True):
            """logits psum[p=(e,b), (g,j)] = sum_d U*v via DVE product +
            PE identity-matmul accumulation over the 32 d slices."""
            if warm:
                pe_warm(20)
            Uq = U[q % 2]
            v4 = (vt[q % 2][:]
                  .rearrange("p (d j) -> p d j", d=D_OUT, j=N_OUT)
                  .unsqueeze(1)
                  .to_broadcast((128, SECG, D_OUT, N_OUT)))
            for sec in range(NSEC):
                slot = slots[sec % 2]
                P = slot[:].rearrange(
                    "p (g d j) -> p g d j", g=SECG, d=D_OUT, j=N_OUT)
                Us = Uq[:, sec * SEC_COLS:(sec + 1) * SEC_COLS].rearrange(
                    "p (g d j) -> p g d j", g=SECG, d=D_OUT, j=N_OUT)
                nc.vector.tensor_tensor(P, Us, v4, AluOp.mult)   # 2x
                for dd in range(D_OUT):
                    nc.tensor.matmul(
                        ps_log[:, sec * SECG * N_OUT:
                               (sec + 1) * SECG * N_OUT],
                        ident[:],
                        P[:, :, dd, :],
                        start=(dd == 0), stop=(dd == D_OUT - 1),
                    )
                if warm and sec % 2 == 1 and sec < NSEC - 1:
                    pe_warm(4)

        def softmax(q, t):
            """c = softmax over j of logits (+ prev-round logits for t=2)."""
            A = algA[q % 2]
            if t == 1:
                nc.scalar.copy(A[:], ps_log[:])
            else:
                nc.vector.tensor_add(A[:], A[:], ps_log[:])
            nc.scalar.activation(ebuf[:], A[:], Act.Exp)
            e3 = ebuf[:].rearrange("p (g j) -> p g j", g=NG, j=N_OUT)
            nc.vector.reduce_sum(Zt[:], e3, axis=Axis.X)
            nc.vector.reciprocal(Zr[:], Zt[:])
            nc.vector.tensor_copy(
                zrep[:].rearrange("p (g j) -> p g j", g=NG, j=N_OUT),
                Zr[:].unsqueeze(2).to_broadcast((128, NG, N_OUT)))
            nc.vector.tensor_tensor(
                cbuf[q % 2][:], ebuf[:], zrep[:], AluOp.mult)

        def weighted_sum(q, t, warm=True):
            """s_partial[b,(d,j)] = sum_i c*U: DVE product (2x) + PE
            ones-matmul reduction over (e-partitions, g-psum-accum)."""
            if warm:
                pe_warm(20)
            Uq = U[q % 2]
            cq = cbuf[q % 2]
            for sec in range(NSEC):
                slot = slots[sec % 2]
                P = slot[:].rearrange(
                    "p (g d j) -> p g d j", g=SECG, d=D_OUT, j=N_OUT)
                Us = Uq[:, sec * SEC_COLS:(sec + 1) * SEC_COLS].rearrange(
                    "p (g d j) -> p g d j", g=SECG, d=D_OUT, j=N_OUT)
                c4 = (cq[:, sec * SECG * N_OUT:(sec + 1) * SECG * N_OUT]
                      .rearrange("p (g j) -> p g j", g=SECG, j=N_OUT)
                      .unsqueeze(2)
                      .to_broadcast((128, SECG, D_OUT, N_OUT)))
                nc.vector.tensor_tensor(P, Us, c4, AluOp.mult)   # 2x
                for g in range(SECG):
                    for half in range(2):
                        nc.tensor.matmul(
                            ps_ws[0:BC, half * 512:(half + 1) * 512],
                            ones16[:],
                            slot[:, g * JD + half * 512:
                                 g * JD + half * 512 + 512],
                            start=(sec == 0 and g == 0),
                            stop=(sec == NSEC - 1 and g == SECG - 1),
                        )
            nc.scalar.copy(s_sb[0:BC, :], ps_ws[0:BC, :])
            ar_in, ar_out = ar_bufs[(q, t)]
            nc.gpsimd.dma_start(ar_in[:], s_sb[0:BC, :])
            nc.gpsimd.collective_compute(
                "AllReduce", AluOp.add, replica_groups=rg,
                ins=[ar_in.opt()], outs=[ar_out.opt()],
            )
            return ar_out

        def squash_core(rows, out_tt):
            """Common squash tail: rows = partition count holding s in
            s_rep; out_tt(s3, f3) emits the final multiply."""
            nc.scalar.square(p2[0:rows, :], s_rep[0:rows, :])
            p3 = p2[0:rows, :].rearrange("p (d j) -> p j d", d=D_OUT, j=N_OUT)
            nc.vector.reduce_sum(s2[0:rows, :], p3, axis=Axis.X)
            nc.vector.tensor_scalar_add(den[0:rows, :], s2[0:rows, :],
                                        1.0 + EPS)
            nc.vector.tensor_scalar_add(lns[0:rows, :], s2[0:rows, :], EPS)
            ii = lns[0:rows, :].bitcast(mybir.dt.int32)
            nc.vector.tensor_scalar(
                ibuf[0:rows, :], ii, 1, None,
                mybir.AluOpType.logical_shift_right)
            nc.vector.tensor_scalar(
                ibuf[0:rows, :], ibuf[0:rows, :], 0x5F3759DF, -1,
                mybir.AluOpType.subtract, mybir.AluOpType.mult)
            y0 = ibuf[0:rows, :].bitcast(f32)
            nc.vector.tensor_mul(rsq[0:rows, :], y0, y0)
            nc.vector.tensor_mul(rsq[0:rows, :], rsq[0:rows, :],
                                 lns[0:rows, :])
            nc.vector.tensor_scalar(
                rsq[0:rows, :], rsq[0:rows, :], -0.5, 1.5,
                mybir.AluOpType.mult, mybir.AluOpType.add)
            nc.vector.tensor_mul(rsq[0:rows, :], rsq[0:rows, :], y0)
            nc.vector.tensor_mul(fsc[0:rows, :], rsq[0:rows, :],
                                 rsq[0:rows, :])
            nc.vector.tensor_mul(fsc[0:rows, :], fsc[0:rows, :],
                                 lns[0:rows, :])
            nc.vector.tensor_scalar(
                fsc[0:rows, :], fsc[0:rows, :], -0.5, 1.5,
                mybir.AluOpType.mult, mybir.AluOpType.add)
            nc.vector.tensor_mul(rsq[0:rows, :], rsq[0:rows, :],
                                 fsc[0:rows, :])
            nc.vector.reciprocal(rinv[0:rows, :], den[0:rows, :])
            nc.vector.tensor_mul(fsc[0:rows, :], rsq[0:rows, :],
                                 rinv[0:rows, :])
            nc.vector.tensor_mul(fsc[0:rows, :], fsc[0:rows, :],
                                 s2[0:rows, :])
            out_tt()

        def squash_v0():
            """v0 = squash(AllReduce(s0)) for the full batch at
            [p=(rep2,b64)]."""
            for r in range(2):
                nc.gpsimd.dma_start(
                    s_rep[64 * r:64 * r + 64, :], ar0_out[:])

            def tt():
                s3 = s_rep[:].rearrange("p (d j) -> p d j", d=D_OUT, j=N_OUT)
                f3 = fsc[:].unsqueeze(1).to_broadcast((128, D_OUT, N_OUT))
                v3 = v0all[:].rearrange("p (d j) -> p d j", d=D_OUT, j=N_OUT)
                nc.vector.tensor_tensor(v3, s3, f3, AluOp.mult)
            squash_core(128, tt)

        def rep_v0(q):
            """vt[q%2] <- per-chunk (e8,b16)-replicated slice of v0all."""
            dst = vt[q % 2]
            for e in range(8):
                nc.gpsimd.dma_start(
                    dst[16 * e:16 * e + 16, :],
                    v0all[q * BC:(q + 1) * BC, :])

        def rep_s(ar_out):
            """Replicate an AllReduced [16,1024] s into s_rep's 8 e-groups
            (Pool queue; emit right after the producing collective so the
            queue never head-blocks)."""
            for e in range(8):
                nc.gpsimd.dma_start(
                    s_rep[16 * e:16 * e + 16, :], ar_out[:])

        def squash_round(q, ar_out, reps_done=False):
            """v_{t} for chunk q from its AllReduced s, into vt[q%2]."""
            if not reps_done:
                rep_s(ar_out)

            def tt():
                s3 = s_rep[:].rearrange("p (d j) -> p d j", d=D_OUT, j=N_OUT)
                f3 = fsc[:].unsqueeze(1).to_broadcast((128, D_OUT, N_OUT))
                v3 = vt[q % 2][:].rearrange(
                    "p (d j) -> p d j", d=D_OUT, j=N_OUT)
                nc.vector.tensor_tensor(v3, s3, f3, AluOp.mult)
            squash_core(128, tt)

        def squash_final(q, ar_out):
            """Final v for chunk q -> v_out rows, reference layout."""
            for e in range(8):
                nc.gpsimd.dma_start(
                    s_rep[16 * e:16 * e + 16, :], ar_out[:])

            def tt():
                vf = v_fin[:].rearrange("p (j d) -> p d j", j=N_OUT, d=D_OUT)
                nc.vector.tensor_tensor(
                    vf,
                    s_rep[0:BC, :].rearrange(
                        "p (d j) -> p d j", d=D_OUT, j=N_OUT),
                    fsc[0:BC, :].unsqueeze(1).to_broadcast(
                        (BC, D_OUT, N_OUT)),
                    AluOp.mult)
                nc.gpsimd.dma_start(v_out[q * BC:(q + 1) * BC, :], v_fin[:])
            squash_core(BC, tt)

        def round_(q, t):
            agreement(q, t)
            softmax(q, t)
            return weighted_sum(q, t)

        # ------------------------------------------------------ emission
        # S0: warm PE, stream W once for chunk 0 while accumulating s0 for
        # the full batch; AllReduce s0; squash v0.
        # CC cold-start warmup: a tiny AllReduce enqueued at t=0 absorbs
        # the ~40us first-collective spin-up while S0 computes.
        nc.gpsimd.dma_start(dum_in[:], s_sb[0:16, 0:16])
        nc.gpsimd.collective_compute(
            "AllReduce", AluOp.add, replica_groups=rg,
            ins=[dum_in.opt()], outs=[dum_out.opt()],
        )
        warmup()
        phase1_fused01()
        nc.scalar.copy(s_sb[:], ps_ws[:])
        nc.gpsimd.dma_start(ar0_in[:], s_sb[:])
        nc.gpsimd.collective_compute(
            "AllReduce", AluOp.add, replica_groups=rg,
            ins=[ar0_in.opt()], outs=[ar0_out.opt()],
        )
        squash_v0()
        rep_v0(0)
        rep_v0(1)

        # S1: t1c0 (U1 already filled by the fused S0 pass)
        ar_c0t1 = round_(0, 1)
        # S2: t1c1
        ar_c1t1 = round_(1, 1)
        # S3: t2c0
        squash_round(0, ar_c0t1)
        ar_c0t2 = round_(0, 2)
        # S4: t2c1 (phase1 c2 overlaps; U0 free after t2c0)
        squash_round(1, ar_c1t1)
        rep_v0(2)
        phase1(2, 0, 16)
        agreement(1, 2, warm=False)
        softmax(1, 2)
        phase1(2, 16, NG)
        ar_c1t2 = weighted_sum(1, 2, warm=False)
        # S5: t1c2 (phase1 c3 overlaps; U1 free after t2c1)
        squash_final(0, ar_c0t2)
        rep_v0(3)
        phase1(3, 0, 16)
        agreement(2, 1, warm=False)
        softmax(2, 1)
        phase1(3, 16, NG)
        ar_c2t1 = weighted_sum(2, 1, warm=False)
        # S6: t1c3
        squash_final(1, ar_c1t2)
        ar_c3t1 = round_(3, 1)
        # S7: t2c2
        squash_round(2, ar_c2t1)
        ar_c2t2 = round_(2, 2)
        # S8: t2c3
        squash_round(3, ar_c3t1)
        ar_c3t2 = round_(3, 2)
        # tail
        squash_final(2, ar_c2t2)
        squash_final(3, ar_c3t2)

    _fix_sync_waits(nc)
    return nc


# ---------------------------------------------------------------- host prep
def _prep_inputs(x, W):
    """Per-core input maps.

    Local capsule l = g*8 + e (g in [0,32), e in [0,8)).
    SBUF rows r128 = e*16 + k.
      wt[(e,k); g*1024 + d*32 + j]         = W[l(g,e), j, d, k]
      xin[(e',k); c*4096 + g*128 + e*16+bb] = [e==e'] x[c*16+bb, l(g,e), k]
      xd[(e,k); g*64 + b]                  = x[b, l(g,e), k] / 32
    """
    import jax.numpy as jnp

    def tobf(a):
        return np.asarray(jnp.asarray(a).astype(jnp.bfloat16))

    in_maps = []
    ident = tobf(np.eye(128, dtype=np.float32))
    ones16 = np.zeros((128, BC), np.float32)
    for p in range(128):
        ones16[p, p % BC] = 1.0
    ones16 = tobf(ones16)
    for c in range(N_CORES):
        xi = np.asarray(x[:, c * I_LOC:(c + 1) * I_LOC, :])   # [B, 256, 16]
        wi = np.asarray(W[0, c * I_LOC:(c + 1) * I_LOC])      # [256, 32, 32, 16]

        w5 = wi.reshape(NG, 8, N_OUT, D_OUT, D_IN)            # g,e,j,d,k
        wt = np.transpose(w5, (1, 4, 0, 3, 2)).reshape(128, UCOLS)

        x5 = xi.reshape(NCHUNK, BC, NG, 8, D_IN)              # c,bb,g,e,k
        xin = np.zeros((8, D_IN, NCHUNK, NG, 8, BC), np.float32)
        for e in range(8):
            xin[e, :, :, :, e, :] = np.transpose(
                x5[:, :, :, e, :], (3, 0, 2, 1))
        xin = xin.reshape(128, NCHUNK * NG * 128)

        xd = (np.transpose(xi.reshape(B, NG, 8, D_IN),
                           (2, 3, 1, 0)) / 32.0).reshape(128, NG * B)

        in_maps.append({
            "wt": tobf(np.ascontiguousarray(wt)),
            "xin": tobf(np.ascontiguousarray(xin)),
            "xd": tobf(np.ascontiguousarray(xd)),
            "ident": ident,
            "ones16": ones16,
        })
    return in_maps


_cached = {}


def _get_program():
    if "nc" not in _cached:
        _cached["nc"] = _build_program()
    return _cached["nc"]


def kernel(x, W):
    x = np.asarray(x, dtype=np.float32)
    W = np.asarray(W, dtype=np.float32)
    nc = _get_program()
    in_maps = _prep_inputs(x, W)
    res = bass_utils.run_bass_kernel_spmd(
        nc, in_maps, core_ids=list(range(N_CORES))
    )
    v = res.results[0]["v"].reshape(B, N_OUT, D_OUT)
    return v.astype(np.float32)
